# revision 11
# baseline (speedup 1.0000x reference)
"""DigitCaps dynamic-routing kernel for 8 trn2 NeuronCores.

Math (reference):
    u_hat[b,r,c,o] = sum_i W[r,c,o,i] * x[b,r,i]        # never materialized!
    3 routing iters:
        c_ij = softmax(b_ij, axis=r)                     # (R, C)
        s    = einsum('rc,brco->bco', c_ij, u_hat)
        v    = squash(s)  elementwise: s|s|/(1+s^2)
        b_ij += mean_b <u_hat[b,r,c,:], v[b,c,:]>

Key restructuring:
  * s[b,(c,o)] = sum_{k=(r,i)} X[b,k] * (exp(b_ij[r,c]) * Wp[k,(c,o)]) / D[c]
    with Wp[(r,i),(c,o)] = W[r,c,o,i] and D[c] = sum_r exp(b_ij[r,c]).
    The softmax normalizer D commutes through the contraction, so each
    routing iteration needs exactly ONE AllReduce (partial s-tilde + partial D).
  * agreement: m[r,c] = (1/B) sum_{i,o} Wp[(r,i),(c,o)] * G[(r,i),(c,o)]
    with G = X^T V  (contraction over batch) -- a second dense matmul.
  * Routes are sharded 8-way (144 routes / core, K=1152 contraction rows);
    batch is replicated. The only cross-core traffic is the per-iteration
    AllReduce of (256x160 partial s + 10 partial D) = ~160KB.
  * The final iteration's AllReduce is folded into the host-side gather:
    each core emits its raw partial s-tilde and partial D; the host sums,
    normalizes and applies the squash.

Matmuls run as float32r (fp32 bits, full-rate PE streaming) with the moving
free dim padded to 256; set _USE_F32R=False for plain fp32 (4 cyc/row).
"""

import numpy as np

import concourse.bass as bass
import concourse.mybir as mybir
from concourse import bacc, tile
from concourse.bass_utils import run_bass_kernel_spmd

B, R, C, O, I = 256, 1152, 10, 16, 8
NCORES = 8
RL = R // NCORES            # 144 routes per core
KL = RL * I                 # 1152 contraction rows per core
KT = KL // 128              # 9 K-tiles of 128
CO = C * O                  # 160
MB = B // 128               # 2 batch blocks of 128
NITER = 3

_USE_F32R = False
NPAD = 256 if _USE_F32R else CO   # moving-dim padding for full-rate f32r

F32 = mybir.dt.float32
F32R = mybir.dt.float32r

_CACHE = {}


def _mm_ap(ap):
    """Bitcast a matmul operand to float32r when enabled."""
    return ap.bitcast(F32R) if _USE_F32R else ap


def _free_bcast(ap, dims):
    """Manual AP with the partition dim kept and custom free dims
    (steps in elements, step 0 = broadcast)."""
    return bass.AP(ap.tensor, ap.offset, [list(ap.ap[0])] + [list(d) for d in dims])


def _build_nc():
    nc = bacc.Bacc(
        trn_type="TRN2",
        target_bir_lowering=False,
        debug=False,
        num_devices=NCORES,
    )

    xt_d = nc.dram_tensor("xt", [KL, B], F32, kind="ExternalInput")      # (r,i) x b
    xn_d = nc.dram_tensor("xn", [B, KL], F32, kind="ExternalInput")      # b x (r,i)
    wt_d = nc.dram_tensor("wt", [KL, NPAD], F32, kind="ExternalInput")   # padded Wp
    jm_d = nc.dram_tensor("jm", [128, 128], F32, kind="ExternalInput")   # kron(I16,ones8)/B
    oc_d = nc.dram_tensor("oc", [128, 1], F32, kind="ExternalInput")     # 1/I column
    outs_d = nc.dram_tensor("out_s", [MB, 128, CO], F32, kind="ExternalOutput")
    outd_d = nc.dram_tensor("out_d", [C, 1], F32, kind="ExternalOutput")

    with tile.TileContext(nc) as tc:
        _body(tc, xt_d, xn_d, wt_d, jm_d, oc_d, outs_d, outd_d)
    nc.compile()
    return nc


def _body(tc, xt_d, xn_d, wt_d, jm_d, oc_d, outs_d, outd_d):
    nc = tc.nc
    ts = bass.ts

    with (
        tc.tile_pool(name="sb", bufs=1) as sb,
        tc.tile_pool(name="pss", bufs=1, space="PSUM") as pss,
        tc.tile_pool(name="psg", bufs=3, space="PSUM") as psg,
        tc.tile_pool(name="psx", bufs=1, space="PSUM") as psx,
        tc.tile_pool(name="dram", bufs=1, space="DRAM") as dram,
    ):
        # ---- persistent SBUF tensors ----
        xt_s = sb.tile([128, KT * B], F32)        # lhsT for s-matmul
        xn_s = sb.tile([128, MB * KL], F32)       # lhsT for G-matmul
        wt_s = sb.tile([128, KT * NPAD], F32)     # Wp (padded)
        wp_s = sb.tile([128, KT * NPAD], F32)     # exp(b)-scaled Wp
        jm_s = sb.tile([128, 128], F32)
        oc_s = sb.tile([128, 1], F32)
        vv_s = sb.tile([128, MB * NPAD], F32)     # squashed v (padded)
        s_s = sb.tile([128, MB * CO], F32)        # summed s-tilde
        b_s = sb.tile([128, KT * C], F32)         # b_ij expanded over i
        ct_s = sb.tile([128, KT * C], F32)        # exp(b_ij)
        cs_s = sb.tile([128, C], F32)             # sum_t exp(b)
        p_s = sb.tile([128, KT * CO], F32)        # Wp .* G scratch
        mio_s = sb.tile([128, KT * C], F32)       # sum_o (Wp .* G)
        a2_s = sb.tile([128, MB * CO], F32)       # s^2
        t2_s = sb.tile([128, MB * CO], F32)       # s^2 + D^2
        r2_s = sb.tile([128, MB * CO], F32)       # 1/(s^2+D^2)
        ab_s = sb.tile([128, MB * CO], F32)       # |s|
        nn_s = sb.tile([128, MB * CO], F32)       # s*|s|
        dbc_s = sb.tile([128, C], F32)            # D broadcast over partitions
        dsq_s = sb.tile([128, C], F32)            # D^2
        ev_s = sb.tile([128, MB * CO], F32)       # PSUM->SBUF s-tilde evac
        dv_s = sb.tile([C, 1], F32)               # PSUM->SBUF D evac

        # ---- DRAM bounce buffers for the collectives ----
        cc0_in = dram.tile([MB, 128, CO], F32)
        cc0_out = dram.tile([MB, 128, CO], F32)
        CC1N = MB * 128 * CO + C
        cc1_in = dram.tile([CC1N], F32)
        cc1_out = dram.tile([CC1N], F32)

        # ---- loads ----
        nc.sync.dma_start(
            out=xt_s.rearrange("p (t b) -> p t b", t=KT),
            in_=xt_d.ap().rearrange("(t p) b -> p t b", p=128),
        )
        nc.sync.dma_start(
            out=xn_s.rearrange("p (m k) -> p m k", m=MB),
            in_=xn_d.ap().rearrange("(m p) k -> p m k", p=128),
        )
        nc.sync.dma_start(
            out=wt_s.rearrange("p (t f) -> p t f", t=KT),
            in_=wt_d.ap().rearrange("(t p) f -> p t f", p=128),
        )
        nc.sync.dma_start(out=jm_s[:], in_=jm_d.ap())
        nc.sync.dma_start(out=oc_s[:], in_=oc_d.ap())

        nc.vector.memset(wp_s[:], 0.0)
        nc.vector.memset(vv_s[:], 0.0)
        nc.vector.memset(b_s[:], 0.0)

        xt3 = xt_s.rearrange("p (t b) -> p t b", t=KT)
        xn3 = xn_s.rearrange("p (m k) -> p m k", m=MB)
        wt3 = wt_s.rearrange("p (t f) -> p t f", t=KT)
        wp3 = wp_s.rearrange("p (t f) -> p t f", t=KT)
        vv3 = vv_s.rearrange("p (m f) -> p m f", m=MB)
        s3 = s_s.rearrange("p (m f) -> p m f", m=MB)

        ev3 = ev_s.rearrange("p (m f) -> p m f", m=MB)

        def stilde_matmul(w3):
            """18 accumulating matmuls -> evacuated into ev_s (128, MB*CO)."""
            sp = [
                pss.tile([128, NPAD], F32, name=f"sps{m}", tag=f"sps{m}")
                for m in range(MB)
            ]
            for t in range(KT):
                for m in range(MB):
                    nc.tensor.matmul(
                        sp[m][:],
                        lhsT=_mm_ap(xt3[:, t, ts(m, 128)]),
                        rhs=_mm_ap(w3[:, t, :]),
                        start=(t == 0),
                        stop=(t == KT - 1),
                    )
            for m in range(MB):
                nc.scalar.activation(
                    ev3[:, m, :], sp[m][:, 0:CO],
                    mybir.ActivationFunctionType.Copy,
                )

        def squash(d_const=None):
            """vv <- squash(s / D): v = s*|s| / (D^2 + s^2).

            d_const: float D for iteration 0; None -> use dbc_s/dsq_s."""
            nc.scalar.activation(a2_s[:], s_s[:], mybir.ActivationFunctionType.Square)
            if d_const is not None:
                nc.scalar.activation(
                    t2_s[:], a2_s[:], mybir.ActivationFunctionType.Copy,
                    bias=float(d_const) ** 2,
                )
            else:
                nc.scalar.activation(
                    dsq_s[:], dbc_s[:], mybir.ActivationFunctionType.Square
                )
                nc.vector.tensor_add(
                    t2_s.rearrange("p (m c o) -> p m c o", m=MB, c=C),
                    a2_s.rearrange("p (m c o) -> p m c o", m=MB, c=C),
                    _free_bcast(dsq_s, [[0, MB], [1, C], [0, O]]),
                )
            nc.vector.reciprocal(r2_s[:], t2_s[:])
            nc.scalar.activation(ab_s[:], s_s[:], mybir.ActivationFunctionType.Abs)
            nc.vector.tensor_mul(nn_s[:], s_s[:], ab_s[:])
            # write only the CO live columns of each padded v block
            nc.vector.tensor_mul(
                _free_bcast(vv_s, [[NPAD, MB], [1, CO]]),
                nn_s.rearrange("p (m f) -> p m f", m=MB),
                r2_s.rearrange("p (m f) -> p m f", m=MB),
            )

        def agreement():
            """G = X^T V; m = (1/B) sum_io Wp.*G; b += m; ct = exp(b); cs, D."""
            for t in range(KT):
                g_ps = psg.tile([128, NPAD], F32, name="gps", tag="gps")
                for m in range(MB):
                    nc.tensor.matmul(
                        g_ps[:],
                        lhsT=_mm_ap(xn3[:, m, ts(t, 128)]),
                        rhs=_mm_ap(vv3[:, m, :]),
                        start=(m == 0),
                        stop=(m == MB - 1),
                    )
                nc.vector.tensor_mul(
                    p_s[:, ts(t, CO)], wt3[:, t, 0:CO], g_ps[:, 0:CO]
                )
            nc.vector.reduce_sum(
                mio_s[:],
                p_s.rearrange("p (t c o) -> p t c o", t=KT, c=C),
                axis=mybir.AxisListType.X,
            )
            me_ps = psx.tile([128, KT * C], F32, name="meps", tag="meps")
            nc.tensor.matmul(
                me_ps[:], lhsT=jm_s[:], rhs=mio_s[:], start=True, stop=True
            )
            nc.vector.tensor_add(b_s[:], b_s[:], me_ps[:])
            nc.scalar.activation(ct_s[:], b_s[:], mybir.ActivationFunctionType.Exp)
            nc.vector.reduce_sum(
                cs_s[:],
                _free_bcast(ct_s, [[1, C], [C, KT]]),
                axis=mybir.AxisListType.X,
            )
            d_ps = psx.tile([C, 1], F32, name="dps", tag="dps")
            nc.tensor.matmul(d_ps[:], lhsT=cs_s[:], rhs=oc_s[:], start=True, stop=True)
            nc.scalar.activation(
                dv_s[:], d_ps[:], mybir.ActivationFunctionType.Copy
            )
            # W' = Wp * exp(b)   (broadcast exp(b) over o; pad cols stay 0)
            nc.vector.tensor_mul(
                _free_bcast(wp_s, [[NPAD, KT], [16, C], [1, O]]),
                _free_bcast(wt_s, [[NPAD, KT], [16, C], [1, O]]),
                _free_bcast(ct_s, [[C, KT], [1, C], [0, O]]),
            )
            return d_ps

        # ================= iteration 0 =================
        stilde_matmul(wt3)
        nc.sync.dma_start(out=cc0_in.rearrange("m p f -> p m f"), in_=ev3)
        nc.gpsimd.collective_compute(
            "AllReduce",
            mybir.AluOpType.add,
            replica_groups=[list(range(NCORES))],
            ins=[cc0_in.opt()],
            outs=[cc0_out.opt()],
        )
        nc.sync.dma_start(out=s3, in_=cc0_out.rearrange("m p f -> p m f"))
        squash(d_const=float(R))           # D0 = R (softmax of zeros)
        agreement()                        # -> b1, exp(b1), D1 partial

        # ================= iteration 1 =================
        stilde_matmul(wp3)
        nc.sync.dma_start(
            out=cc1_in[0:MB * 128 * CO].rearrange("(m p f) -> p m f", m=MB, p=128),
            in_=ev3,
        )
        nc.sync.dma_start(out=cc1_in[MB * 128 * CO:CC1N], in_=dv_s[:])
        nc.gpsimd.collective_compute(
            "AllReduce",
            mybir.AluOpType.add,
            replica_groups=[list(range(NCORES))],
            ins=[cc1_in.opt()],
            outs=[cc1_out.opt()],
        )
        nc.sync.dma_start(
            out=s3,
            in_=cc1_out[0:MB * 128 * CO].rearrange("(m p f) -> p m f", m=MB, p=128),
        )
        # D row from DRAM, replicated across all 128 partitions
        nc.sync.dma_start(
            out=dbc_s[:],
            in_=bass.AP(cc1_out.tensor, cc1_out.offset + MB * 128 * CO, [[0, 128], [1, C]]),
        )
        squash()
        agreement()                        # -> b2, exp(b2), D2 partial

        # ================= iteration 2 =================
        stilde_matmul(wp3)
        nc.sync.dma_start(out=outs_d.ap().rearrange("m p f -> p m f"), in_=ev3)
        nc.sync.dma_start(out=outd_d.ap(), in_=dv_s[:])


def _prep_inputs(x, W):
    x = np.ascontiguousarray(np.asarray(x, np.float32))
    W = np.asarray(W, np.float32)
    Wp = np.ascontiguousarray(W.transpose(0, 3, 1, 2).reshape(R * I, C * O))
    Wpad = np.zeros((R * I, NPAD), np.float32)
    Wpad[:, :CO] = Wp
    jm = (np.kron(np.eye(16, dtype=np.float32), np.ones((8, 8), np.float32)) / B
          ).astype(np.float32)
    oc = np.full((128, 1), 1.0 / I, np.float32)
    in_maps = []
    for k in range(NCORES):
        xs = x[:, k * RL:(k + 1) * RL, :].reshape(B, KL)
        in_maps.append({
            "xt": np.ascontiguousarray(xs.T),
            "xn": np.ascontiguousarray(xs),
            "wt": np.ascontiguousarray(Wpad[k * KL:(k + 1) * KL]),
            "jm": jm,
            "oc": oc,
        })
    return in_maps


def _postprocess(results):
    s = np.zeros((MB, 128, CO), np.float64)
    D = np.zeros((C, 1), np.float64)
    for r in results:
        s += r["out_s"].astype(np.float64)
        D += r["out_d"].astype(np.float64)
    s = s.reshape(B, C, O)
    sn = s / D.reshape(C)[None, :, None]
    v = sn * np.abs(sn) / (1.0 + sn * sn)
    return v[..., None].astype(np.float32)


def _get_nc():
    if "nc" not in _CACHE:
        _CACHE["nc"] = _build_nc()
    return _CACHE["nc"]


def run_on_hw(x, W, **kw):
    """Run the bass kernel on the 8 cores; kw forwarded (e.g. trace=True)."""
    nc = _get_nc()
    in_maps = _prep_inputs(x, W)
    res = run_bass_kernel_spmd(nc, in_maps, core_ids=list(range(NCORES)), **kw)
    return _postprocess(res.results), res


def kernel(x, W):
    out, _ = run_on_hw(x, W)
    return out
